# revision 3
# baseline (speedup 1.0000x reference)
"""Trainium2 Bass kernel v3 for nn_AugmentWithTrace (gnn_message_passing).

Reference computation:
    g = trace_pool[neighbor_idx]                       # [T, K, D] gather
    s = MLP3(g)                                        # per-row scores
    attn = masked_softmax_k(s)                         # over K=8 neighbors
    out = einsum('tk,tkd->td', attn, g)                # [T, D]

Key restructure: the MLP score depends only on the pool row, so scores are
computed once per pool row (131072 rows sharded 8 ways = 16384 rows/core,
half the FLOPs of the per-(t,k) formulation, zero PE transposes), shared
via a sliced bf16 AllGather, then gathered per (t,k) at 2B granularity.

Per core:
  Phase A: 32 col-blocks of 512 pool rows from host-pretransposed bf16
    poolTh; L1 (ACT relu epilogue) / L2 (DVE relu epilogue) as [128,512]
    matmuls; L3 as 1-col matmuls (lhsT=h2 slice, rhs=w3) so scores land
    transposed in PSUM [128, 32] per quarter; per-quarter DMA + AllGather
    slice so collectives overlap phase A.
  Phase B: 8 sliced indirect gathers of neighbor rows (bf16, no deps on
    phase A -> overlap), per-group score gather + masked softmax, then
    weighted sum on the PE: out_chunk = sum_k diag(a_k) @ g_k accumulated
    in PSUM, drained to bf16 by ACT, host upcasts output to f32.
"""

import sys

if "/opt/trn_rl_repo" not in sys.path:
    sys.path.insert(0, "/opt/trn_rl_repo")

import numpy as np
import ml_dtypes

T, K, D, N_POOL = 32768, 8, 256, 131072
N_CORES = 8
T_LOC = T // N_CORES          # 4096 tokens per core
N_CHUNK = T_LOC // 128        # 32 chunks of 128 tokens
N_SHARD = N_POOL // N_CORES   # 16384 pool rows per core
N_BLK = N_SHARD // 512        # 32 phase-A col blocks of 512 rows
N_GRP = N_CHUNK // 4          # 8 groups of 4 chunks (phase B granularity)
N_CC = 1                      # collective slices

_CACHE = {}


def _build_kernel(stage=9):
    import concourse.bass as bass
    import concourse.bacc as bacc
    import concourse.mybir as mybir
    import concourse.tile as tile
    from concourse.masks import make_identity

    f32 = mybir.dt.float32
    bf16 = mybir.dt.bfloat16
    fp8 = mybir.dt.float8e4
    i32 = mybir.dt.int32

    nc = bacc.Bacc("TRN2", target_bir_lowering=False, debug=False,
                   num_devices=N_CORES)

    pool_d = nc.declare_dram_parameter("pool", [N_POOL, D], bf16, isOutput=False)
    poolth_d = nc.declare_dram_parameter("poolth", [128, N_BLK * 1024], bf16,
                                         isOutput=False)
    idx_d = nc.declare_dram_parameter("idx", [128, N_CHUNK * K], i32, isOutput=False)
    idx2_d = nc.declare_dram_parameter("idx2", [128, N_CHUNK * K], i32, isOutput=False)
    maskc_d = nc.declare_dram_parameter("maskc", [128, N_CHUNK * K], f32, isOutput=False)
    w1_d = nc.declare_dram_parameter("w1", [128, 512], bf16, isOutput=False)
    w2_d = nc.declare_dram_parameter("w2", [128, 512], bf16, isOutput=False)
    w3_d = nc.declare_dram_parameter("w3", [128, 2], bf16, isOutput=False)
    b1_d = nc.declare_dram_parameter("b1c", [128, 2], f32, isOutput=False)
    b2_d = nc.declare_dram_parameter("b2c", [128, 2], f32, isOutput=False)
    out_d = nc.declare_dram_parameter("out", [T_LOC, D], bf16, isOutput=True)

    relu = mybir.ActivationFunctionType.Relu
    expf = mybir.ActivationFunctionType.Exp
    mult = mybir.AluOpType.mult

    with tile.TileContext(nc) as tc:
        with (
            tc.tile_pool(name="const", bufs=1) as cp,
            tc.tile_pool(name="gall", bufs=1) as gp,
            tc.tile_pool(name="xin", bufs=4) as xp,
            tc.tile_pool(name="h1p", bufs=3) as h1p,
            tc.tile_pool(name="h2p", bufs=3) as h2p,
            tc.tile_pool(name="scr", bufs=1) as scp,
            tc.tile_pool(name="sml", bufs=6) as sp,
            tc.tile_pool(name="diag", bufs=3) as dgp,
            tc.tile_pool(name="outp", bufs=3) as op_,
            tc.tile_pool(name="pl1", bufs=2, space="PSUM") as pl1,
            tc.tile_pool(name="pl2", bufs=2, space="PSUM") as pl2,
            tc.tile_pool(name="pl3", bufs=1, space="PSUM") as pl3,
            tc.tile_pool(name="pw", bufs=1, space="PSUM") as pwp,
            tc.tile_pool(name="dram", bufs=1, space="DRAM") as dp,
        ):
            # ---- constants loaded once ----
            idx_t = cp.tile([128, N_CHUNK * K], i32)
            nc.sync.dma_start(out=idx_t[:], in_=idx_d[:])
            idx2_t = cp.tile([128, N_CHUNK * K], i32)
            nc.sync.dma_start(out=idx2_t[:], in_=idx2_d[:])
            maskc_t = cp.tile([128, N_CHUNK * K], f32)
            nc.sync.dma_start(out=maskc_t[:], in_=maskc_d[:])
            w1_t = cp.tile([128, 512], bf16)
            nc.sync.dma_start(out=w1_t[:], in_=w1_d[:])
            w2_t = cp.tile([128, 512], bf16)
            nc.sync.dma_start(out=w2_t[:], in_=w2_d[:])
            w3_t = cp.tile([128, 2], bf16)
            nc.sync.dma_start(out=w3_t[:], in_=w3_d[:])
            b1_t = cp.tile([128, 2], f32)
            nc.sync.dma_start(out=b1_t[:], in_=b1_d[:])
            b2_t = cp.tile([128, 2], f32)
            nc.sync.dma_start(out=b2_t[:], in_=b2_d[:])
            ident = cp.tile([128, 128], bf16)
            make_identity(nc, ident[:])
            zeros = cp.tile([128, 512], f32)
            nc.vector.memset(zeros[:], 0.0)

            # ---- phase B neighbor-row gathers: no deps on phase A, so the
            # DMA engines stream rows while PE runs phase A.  Sliced per
            # group so one instruction doesn't monopolize the DMA engines.
            g_t = gp.tile([128, N_CHUNK * K * D], bf16, name="gall", tag="g")

            def gather_g(grp):
                # HW indirect DMA honors ONE dynamic offset per partition, so
                # issue one call per (chunk, k) column exactly like v1 did
                i0 = grp * 4 * K
                for j in range(4 * K):
                    col = i0 + j
                    nc.gpsimd.indirect_dma_start(
                        out=g_t[:, col * D:(col + 1) * D],
                        out_offset=None,
                        in_=pool_d[:],
                        in_offset=bass.IndirectOffsetOnAxis(
                            ap=idx_t[:, col:col + 1], axis=0),
                    )

            # front-load only half the row gathers; the rest issue after the
            # collective so phase A's xin loads aren't DMA-starved
            for grp in range(N_GRP // 2):
                gather_g(grp)

            # ---- phase A: scores for this core's pool shard ----
            s_my = dp.tile([N_SHARD, 1], bf16, name="s_my")
            s_full = dp.tile([N_POOL, 1], bf16, name="s_full")
            blk_per_cc = N_BLK // N_CC
            for q in range(N_CC):
                psc = pl3.tile([128, 32 * blk_per_cc // 8], f32,
                               name=f"psc{q}", tag="psc")
                for bi in range(blk_per_cc):
                    blk = q * blk_per_cc + bi
                    xin = xp.tile([128, 1024], bf16, name=f"xin{blk}", tag="xin")
                    nc.sync.dma_start(
                        out=xin[:],
                        in_=poolth_d[:, blk * 1024:(blk + 1) * 1024])

                    # L1: both j halves into one PSUM tile, single ACT drain
                    h1 = h1p.tile([128, 1024], bf16, name=f"h1_{blk}", tag="h1")
                    ps1 = pl1.tile([128, 1024], f32, name=f"ps1_{blk}", tag="ps1")
                    for j in range(2):
                        for h in range(2):
                            nc.tensor.matmul(
                                out=ps1[:, j * 512:(j + 1) * 512],
                                lhsT=w1_t[:, h * 256 + j * 128:h * 256 + (j + 1) * 128],
                                rhs=xin[:, h * 512:(h + 1) * 512],
                                start=(h == 0), stop=(h == 1),
                            )
                    for j in range(2):
                        nc.scalar.activation(
                            out=h1[:, j * 512:(j + 1) * 512],
                            in_=ps1[:, j * 512:(j + 1) * 512],
                            func=relu, bias=b1_t[:, j:j + 1], scale=1.0)

                    # L2: DVE relu epilogue (max(z + b2, 0))
                    h2 = h2p.tile([128, 1024], bf16, name=f"h2_{blk}", tag="h2")
                    for j in range(2):
                        ps2 = pl2.tile([128, 512], f32, name=f"ps2_{blk}_{j}",
                                       tag="ps2")
                        for h in range(2):
                            nc.tensor.matmul(
                                out=ps2[:],
                                lhsT=w2_t[:, h * 256 + j * 128:h * 256 + (j + 1) * 128],
                                rhs=h1[:, h * 512:(h + 1) * 512],
                                start=(h == 0), stop=(h == 1),
                            )
                        nc.vector.scalar_tensor_tensor(
                            out=h2[:, j * 512:(j + 1) * 512],
                            in0=ps2[:],
                            scalar=b2_t[:, j:j + 1],
                            in1=zeros[:],
                            op0=mybir.AluOpType.add,
                            op1=mybir.AluOpType.max)

                    # L3 transposed: scores land as psc[:, bi*4 + sub]
                    # (pool row = blk*512 + sub*128 + p)
                    for sub in range(4):
                        for h in range(2):
                            nc.tensor.matmul(
                                out=psc[:, bi * 4 + sub:bi * 4 + sub + 1],
                                lhsT=h2[:, h * 512 + sub * 128:h * 512 + (sub + 1) * 128],
                                rhs=w3_t[:, h:h + 1],
                                start=(h == 0), stop=(h == 1),
                            )

                # store E = exp(s): skips the exp on the phase-B tail
                scq = scp.tile([128, 32 * blk_per_cc // 8], bf16,
                               name=f"scq{q}", tag="ssb")
                nc.scalar.activation(out=scq[:], in_=psc[:], func=expf,
                                     bias=0.0, scale=1.0)
                # flat n (within quarter) = col*128 + p  <->  scq[p, col]
                nc.sync.dma_start(
                    out=s_my[q * (N_SHARD // N_CC):(q + 1) * (N_SHARD // N_CC), :]
                        .rearrange("(c p) o -> p (c o)", p=128),
                    in_=scq[:])
                # s_full layout is [q, r, n] (slice-major) so each collective
                # output is contiguous; score-gather indices are host-remapped
                # to match (idx2).
                nc.gpsimd.collective_compute(
                    "AllGather",
                    mybir.AluOpType.bypass,
                    replica_groups=[list(range(N_CORES))],
                    ins=[s_my[q * (N_SHARD // N_CC):(q + 1) * (N_SHARD // N_CC), :].opt()],
                    outs=[s_full[q * N_CORES * (N_SHARD // N_CC):(q + 1) * N_CORES * (N_SHARD // N_CC), :].opt()],
                )

            # schedule-gate the deferred gathers so the Tile scheduler
            # cannot hoist them into phase A's DMA window
            with tc.tile_wait_until(0.050):
                for grp in range(N_GRP // 2, N_GRP):
                    gather_g(grp)

            # ---- phase B: per group of 4 chunks ----
            for grp in range(N_GRP):
                c0 = grp * 4
                i0 = c0 * K           # first idx column of this group
                # gather bf16 scores for the group's 32 (t, k) columns
                s_g = sp.tile([128, 32], bf16, name=f"sg{grp}", tag="sg")
                for j in range(32):
                    nc.gpsimd.indirect_dma_start(
                        out=s_g[:, j:j + 1],
                        out_offset=None,
                        in_=s_full[:],
                        in_offset=bass.IndirectOffsetOnAxis(
                            ap=idx2_t[:, i0 + j:i0 + j + 1], axis=0),
                    )
                # masked unnormalized weights em = E*mask; normalization
                # (1/z) is folded into the ACT drain as a per-partition scale
                em_g = sp.tile([128, 32], f32, name=f"emg{grp}", tag="em")
                nc.vector.tensor_tensor(
                    out=em_g[:], in0=s_g[:], in1=maskc_t[:, i0:i0 + 32],
                    op=mult)
                # build diag(em) for all 32 (c,k) of the group in one DVE op
                dg = dgp.tile([128, 32 * 128], bf16, name=f"dg{grp}", tag="dg")
                nc.vector.tensor_tensor(
                    out=dg[:].rearrange("p (ck c) -> p ck c", c=128),
                    in0=ident[:].unsqueeze(1).broadcast_to([128, 32, 128]),
                    in1=em_g[:].unsqueeze(2).broadcast_to([128, 32, 128]),
                    op=mult)
                z_g = sp.tile([128, 4], f32, name=f"zg{grp}", tag="z")
                nc.vector.reduce_sum(
                    z_g[:],
                    em_g[:].rearrange("p (c k) -> p c k", k=K),
                    axis=mybir.AxisListType.X)
                nc.vector.tensor_scalar_add(z_g[:], z_g[:], 1e-30)
                r_g = sp.tile([128, 4], f32, name=f"rg{grp}", tag="r")
                nc.vector.reciprocal(out=r_g[:], in_=z_g[:])
                # weighted sum on PE: psw_c = sum_k diag(em_ck) @ g_ck;
                # ACT drain rescales by r_c while casting to bf16
                ob = op_.tile([128, 4 * D], bf16, name=f"ob{grp}", tag="ob")
                for cc in range(4):
                    c = c0 + cc
                    g0 = c * K * D
                    psw = pwp.tile([128, D], f32, name=f"psw{c}", tag="psw")
                    for k in range(K):
                        nc.tensor.matmul(
                            out=psw[:],
                            lhsT=dg[:, (cc * K + k) * 128:(cc * K + k + 1) * 128],
                            rhs=g_t[:, g0 + k * D:g0 + (k + 1) * D],
                            start=(k == 0), stop=(k == K - 1),
                        )
                    nc.scalar.activation(
                        out=ob[:, cc * D:(cc + 1) * D], in_=psw[:],
                        func=mybir.ActivationFunctionType.Copy,
                        bias=0.0, scale=r_g[:, cc:cc + 1])
                nc.sync.dma_start(
                    out=out_d[grp * 512:(grp + 1) * 512, :]
                        .rearrange("(cc p) d -> p cc d", p=128),
                    in_=ob[:].rearrange("p (cc d) -> p cc d", cc=4))

    nc.compile()
    return nc


def _prep_core_inputs(c, pool_bf16, poolth_all, neighbor_idx, mask_f, w_shared):
    t0 = c * T_LOC
    nidx = neighbor_idx[t0:t0 + T_LOC]                     # [T_LOC, K]
    # column (chunk*K + k), partition p -> token chunk*128+p, neighbor k
    idx_arr = np.ascontiguousarray(
        nidx.reshape(N_CHUNK, 128, K).transpose(1, 0, 2).reshape(128, N_CHUNK * K)
    ).astype(np.int32)
    # score-vector layout after sliced AllGather is [q, r, n]:
    # global row r*16384 + q*4096 + n sits at position q*32768 + r*4096 + n
    ssz = N_SHARD // N_CC
    r_ = idx_arr // N_SHARD
    rem = idx_arr % N_SHARD
    q_ = rem // ssz
    n_ = rem % ssz
    idx2_arr = (q_ * (N_CORES * ssz) + r_ * ssz + n_).astype(np.int32)
    mcol = np.ascontiguousarray(
        mask_f[t0:t0 + T_LOC].reshape(N_CHUNK, 128, K)
        .transpose(1, 0, 2).reshape(128, N_CHUNK * K)
    ).astype(np.float32)
    m = {"pool": pool_bf16, "poolth": poolth_all[c], "idx": idx_arr,
         "idx2": idx2_arr, "maskc": mcol}
    m.update(w_shared)
    return m


def _prep_shared_weights(inputs):
    W1 = np.asarray(inputs["W1"], dtype=np.float32)
    W2 = np.asarray(inputs["W2"], dtype=np.float32)
    W3 = np.asarray(inputs["W3"], dtype=np.float32)
    b1 = np.asarray(inputs["b1"], dtype=np.float32)
    b2 = np.asarray(inputs["b2"], dtype=np.float32)
    bfc = lambda x: np.ascontiguousarray(x).astype(ml_dtypes.bfloat16)
    # w[p, h*256 + j*128 + jj] = W[h*128+p, j*128+jj]
    return {
        "w1": bfc(W1.reshape(2, 128, 256).transpose(1, 0, 2).reshape(128, 512)),
        "w2": bfc(W2.reshape(2, 128, 256).transpose(1, 0, 2).reshape(128, 512)),
        "w3": bfc(W3.reshape(2, 128).T),
        "b1c": np.ascontiguousarray(b1.reshape(2, 128).T).astype(np.float32),
        "b2c": np.ascontiguousarray(b2.reshape(2, 128).T).astype(np.float32),
    }


def _prep_poolth(pool_bf16):
    # poolth[c][p, blk*1024 + h*512 + col] = pool[c*N_SHARD + blk*512 + col,
    #                                             h*128 + p]
    per_core = []
    for c in range(N_CORES):
        shard = pool_bf16[c * N_SHARD:(c + 1) * N_SHARD]      # [16384, 256]
        pt = np.ascontiguousarray(
            shard.reshape(N_BLK, 512, 2, 128).transpose(3, 0, 2, 1)
            .reshape(128, N_BLK * 1024))
        per_core.append(pt)
    return per_core


def kernel(trace_pool, neighbor_idx, neighbor_mask, W1, b1, W2, b2, W3, b3):
    # b3 shifts every logit of a token equally -> softmax-invariant; and
    # no-neighbor rows are zeroed regardless.  The kernel ignores it.
    if "nc" not in _CACHE:
        _CACHE["nc"] = _build_kernel()
    nc = _CACHE["nc"]

    from concourse.bass_utils import run_bass_kernel_spmd

    pool_bf16 = np.ascontiguousarray(
        np.asarray(trace_pool, dtype=np.float32)).astype(ml_dtypes.bfloat16)
    poolth_all = _prep_poolth(pool_bf16)
    neighbor_idx = np.asarray(neighbor_idx, dtype=np.int32)
    mask_f = np.asarray(neighbor_mask).astype(np.float32)
    w_shared = _prep_shared_weights(
        {"W1": W1, "b1": b1, "W2": W2, "b2": b2, "W3": W3, "b3": b3}
    )

    in_maps = [
        _prep_core_inputs(c, pool_bf16, poolth_all, neighbor_idx, mask_f,
                          w_shared)
        for c in range(N_CORES)
    ]
    res = run_bass_kernel_spmd(nc, in_maps, core_ids=list(range(N_CORES)))
    out = np.concatenate(
        [np.asarray(res.results[c]["out"], dtype=np.float32)
         for c in range(N_CORES)], axis=0)
    return out
